# revision 11
# baseline (speedup 1.0000x reference)
"""nn_BlockwiseToPixels: per-token MoE routing (16 experts, Linear(256->64)).

Strategy
--------
Data-parallel over batch (4 batches/core x 8 cores). Inside each core's
shard, tokens are grouped by their routed expert (host-side argsort of the
tiny index tensor; segments padded to 128-token multiples), and the shard's
activations are shipped to the device pre-transposed ([D, Ntot]) because the
TensorEngine contracts over the partition axis. The device kernel is then a
pure memory-roofline streaming job: for every 128-token chunk it loads
xT tiles, runs two fp32 matmuls (D=256 split in two K=128 halves,
x-chunk stationary, expert weight moving) accumulating in PSUM, adds the
expert bias during the PSUM->SBUF copy, and streams the token-major result
back to HBM. The expert of every chunk is a compile-time constant (the
segment layout), so there is no on-device routing logic, no select, and
exactly 1x the required FLOPs in exact fp32.

The compiled program depends only on the per-expert segment capacities, so
it is cached across calls.
"""
import os
import sys

sys.path.insert(0, "/opt/trn_rl_repo")

import numpy as np

import concourse.bass as bass
import concourse.mybir as mybir
import concourse.tile as tile
from concourse.bass_utils import run_bass_kernel_spmd

B, T, D, E, P = 32, 8192, 256, 16, 64
N_CORES = 8
BC = B // N_CORES          # batches per core
N_SHARD = BC * T           # tokens per core
CHUNK = 128                # tokens per matmul chunk (PE partition width)
GROUP = 2048               # tokens per DMA group (16 chunks)

# The pinned walrus accepts only ONE sem wait per instruction, while Tile
# emits instructions carrying several. Hoist extra waits onto InstNoOp
# instructions inserted immediately before, on the same engine (the
# sequencer blocks on each in order - semantically identical).


def _split_multi_waits(nc, max_waits=1):
    n_split = 0
    for f in nc.m.functions:
        for bb in f.blocks:
            il = bb.instructions
            i = 0
            while i < len(il):
                inst = il[i]
                si = inst.sync_info
                if si is not None and si.on_wait and len(si.on_wait) > max_waits:
                    waits = list(si.on_wait)
                    extra, keep = waits[:-max_waits], waits[-max_waits:]
                    nops = []
                    for j, w in enumerate(extra):
                        nop = mybir.InstNoOp(
                            name=f"{inst.name}-waitsplit-{j}", ins=[], outs=[]
                        )
                        nop.engine = inst.engine
                        nop.sync_info = mybir.SyncInfo(on_wait=[w], on_update=[])
                        nops.append(nop)
                    si.on_wait = keep
                    il[i:i] = nops
                    i += len(nops)
                    n_split += 1
                i += 1
    return n_split


def _build_program(chunk_expert):
    """Bass program for one core: segmented matmul over pre-sorted xT.

    chunk_expert: tuple of expert ids, one per 128-token chunk (static).

    W-stationary orientation: fp32 matmuls re-load their stationary operand
    on every instruction (no standalone ldweights for fp32), so the moving
    operand is made as wide as possible (up to 512 tokens) to amortize it.
    Output is pixel-major ysT [P, ntot]; the host transposes it back.
    """
    ntot = len(chunk_expert) * CHUNK
    assert ntot % 1024 == 0
    # groups of GROUP tokens, with an optional 1024-token tail
    groups = []
    pos = 0
    while pos < ntot:
        gl = min(GROUP, ntot - pos)
        groups.append((pos, gl))
        pos += gl

    nc = bass.Bass(trn_type="TRN2")
    dt = mybir.dt
    xT = nc.declare_dram_parameter("xT", [D, ntot], dt.float32, isOutput=False)
    Wp = nc.declare_dram_parameter("Wp", [128, E * 2 * P], dt.float32, isOutput=False)
    bT = nc.declare_dram_parameter("bT", [P, E], dt.float32, isOutput=False)
    ysT = nc.declare_dram_parameter("ysT", [P, ntot], dt.float32, isOutput=True)

    with tile.TileContext(nc) as tc:
        with (
            tc.tile_pool(name="consts", bufs=1) as consts,
            tc.tile_pool(name="xtp", bufs=6) as xtp,
            tc.tile_pool(name="yp", bufs=4) as yp,
            tc.tile_pool(name="ps", bufs=8, space="PSUM") as ps,
        ):
            wt = consts.tile([128, E * 2 * P], dt.float32)
            # split so the first runs (lowest experts) unblock early
            for s in range(0, E * 2 * P, 4 * P):
                nc.sync.dma_start(wt[:, s : s + 4 * P], Wp[:, s : s + 4 * P])
            bt = consts.tile([P, E], dt.float32)
            nc.sync.dma_start(bt[:], bT[:])

            for gi, (gof, gl) in enumerate(groups):
                xt0 = xtp.tile([128, GROUP], dt.float32, tag="xt0")
                xt1 = xtp.tile([128, GROUP], dt.float32, tag="xt1")
                # split the first group's loads so the PE can start after the
                # first 512-column piece lands instead of the whole MiB
                step = 512 if gi == 0 else gl
                for s in range(0, gl, step):
                    nc.sync.dma_start(
                        xt0[:, s : s + step], xT[0:128, gof + s : gof + s + step]
                    )
                    nc.sync.dma_start(
                        xt1[:, s : s + step], xT[128:256, gof + s : gof + s + step]
                    )

                yts = yp.tile([P, GROUP], dt.float32, tag="yts")
                # runs of equal expert within 512-aligned blocks (moving
                # operand / PSUM bank limit for fp32 is 512)
                base = gof // CHUNK
                runs = []
                for blk in range(gl // 512):
                    start = blk * 4
                    for c in range(start, start + 4):
                        e = chunk_expert[base + c]
                        if runs and runs[-1][0] == e and runs[-1][1] + runs[-1][2] == c and c != start:
                            runs[-1][2] += 1
                        else:
                            runs.append([e, c, 1])
                for e, c0, ln in runs:
                    off = c0 * CHUNK
                    n = ln * CHUNK
                    pt = ps.tile([P, n], dt.float32, tag="pt")
                    nc.tensor.matmul(
                        pt[:],
                        lhsT=wt[:, (e * 2 + 0) * P : (e * 2 + 1) * P],
                        rhs=xt0[:, off : off + n],
                        start=True,
                        stop=False,
                    )
                    nc.tensor.matmul(
                        pt[:],
                        lhsT=wt[:, (e * 2 + 1) * P : (e * 2 + 2) * P],
                        rhs=xt1[:, off : off + n],
                        start=False,
                        stop=True,
                    )
                    # bias add doubles as the PSUM->SBUF copy
                    nc.vector.tensor_scalar_add(
                        yts[:, off : off + n], pt[:], bt[:, e : e + 1]
                    )
                nc.scalar.dma_start(ysT[:, gof : gof + gl], yts[:, :gl])

    return nc


_cache = {"key": None, "nc": None}
last_exec_time_ns = None


def kernel(x, W, b, block_indices):
    global last_exec_time_ns
    x = np.asarray(x, dtype=np.float32)
    W = np.asarray(W, dtype=np.float32)
    b = np.asarray(b, dtype=np.float32)
    sel = np.asarray(block_indices)
    sel_dtype = sel.dtype
    sel = sel.astype(np.int64)

    x_sh = x.reshape(N_CORES, N_SHARD, D)
    sel_sh = sel.reshape(N_CORES, N_SHARD)

    # per-(core, expert) token lists, grouped by expert via stable argsort
    counts = np.zeros((N_CORES, E), dtype=np.int64)
    ids = []
    for c in range(N_CORES):
        order = np.argsort(sel_sh[c], kind="stable")
        se = sel_sh[c][order]
        bounds = np.searchsorted(se, np.arange(E + 1))
        ids.append([order[bounds[e] : bounds[e + 1]] for e in range(E)])
        counts[c] = bounds[1:] - bounds[:-1]

    # shared static segment layout: capacity per expert = max over cores,
    # rounded up to CHUNK; total rounded up to GROUP
    caps = ((counts.max(axis=0) + CHUNK - 1) // CHUNK * CHUNK).astype(np.int64)
    ntot = int(((caps.sum() + 1023) // 1024) * 1024)
    caps[E - 1] += ntot - caps.sum()
    offs = np.concatenate([[0], np.cumsum(caps)])

    chunk_expert = []
    for e in range(E):
        chunk_expert += [e] * (int(caps[e]) // CHUNK)
    chunk_expert = tuple(chunk_expert)

    if _cache["key"] != chunk_expert:
        nc = _build_program(chunk_expert)
        _split_multi_waits(nc)
        _cache["nc"] = nc
        _cache["key"] = chunk_expert

    # weights: [E, D, P] -> [128, E*2*P] tiles (K-half h of expert e at
    # columns (e*2+h)*P); bias transposed to per-partition columns [P, E]
    Wp = np.ascontiguousarray(
        W.reshape(E, 2, 128, P).transpose(2, 0, 1, 3).reshape(128, E * 2 * P)
    )
    bT = np.ascontiguousarray(b.T)

    in_maps = []
    for c in range(N_CORES):
        # padded sorted order; pad slots replay token 0 (results discarded)
        po = np.zeros(ntot, dtype=np.int64)
        for e in range(E):
            po[offs[e] : offs[e] + counts[c, e]] = ids[c][e]
        xT = np.ascontiguousarray(x_sh[c][po].T)
        in_maps.append({"xT": xT, "Wp": Wp, "bT": bT})

    trace = bool(os.environ.get("BASS_KERNEL_TRACE"))
    res = run_bass_kernel_spmd(
        _cache["nc"], in_maps, list(range(N_CORES)), trace=trace
    )
    last_exec_time_ns = res.exec_time_ns

    out = np.empty((N_CORES, N_SHARD, P), dtype=np.float32)
    for c in range(N_CORES):
        ys = np.ascontiguousarray(res.results[c]["ysT"].T)
        flat = out[c]
        for e in range(E):
            flat[ids[c][e]] = ys[offs[e] : offs[e] + counts[c, e]]
    _ = sel_dtype
    return out.reshape(B, T, P)


# revision 13
# speedup vs baseline: 1.1588x; 1.1588x over previous
"""nn_BlockwiseToPixels: per-token MoE routing (16 experts, Linear(256->64)).

Strategy
--------
Data-parallel over batch (4 batches/core x 8 cores). Inside each core's
shard, tokens are grouped by their routed expert (host-side argsort of the
tiny index tensor; segments padded to 128-token multiples), and the shard's
activations are shipped to the device pre-transposed ([D, Ntot]) because the
TensorEngine contracts over the partition axis. The device kernel is then a
pure memory-roofline streaming job: for every 128-token chunk it loads
xT tiles, runs two fp32 matmuls (D=256 split in two K=128 halves,
x-chunk stationary, expert weight moving) accumulating in PSUM, adds the
expert bias during the PSUM->SBUF copy, and streams the token-major result
back to HBM. The expert of every chunk is a compile-time constant (the
segment layout), so there is no on-device routing logic, no select, and
exactly 1x the required FLOPs in exact fp32.

The compiled program depends only on the per-expert segment capacities, so
it is cached across calls.
"""
import os
import sys

sys.path.insert(0, "/opt/trn_rl_repo")

import numpy as np

import concourse.bass as bass
import concourse.mybir as mybir
import concourse.tile as tile
from concourse.bass_utils import run_bass_kernel_spmd

B, T, D, E, P = 32, 8192, 256, 16, 64
N_CORES = 8
BC = B // N_CORES          # batches per core
N_SHARD = BC * T           # tokens per core
CHUNK = 128                # tokens per matmul chunk (PE partition width)
GROUP = 2048               # tokens per DMA group (16 chunks)

# The pinned walrus accepts only ONE sem wait per instruction, while Tile
# emits instructions carrying several. Hoist extra waits onto InstNoOp
# instructions inserted immediately before, on the same engine (the
# sequencer blocks on each in order - semantically identical).


def _split_multi_waits(nc, max_waits=1):
    n_split = 0
    for f in nc.m.functions:
        for bb in f.blocks:
            il = bb.instructions
            i = 0
            while i < len(il):
                inst = il[i]
                si = inst.sync_info
                if si is not None and si.on_wait and len(si.on_wait) > max_waits:
                    waits = list(si.on_wait)
                    extra, keep = waits[:-max_waits], waits[-max_waits:]
                    nops = []
                    for j, w in enumerate(extra):
                        nop = mybir.InstNoOp(
                            name=f"{inst.name}-waitsplit-{j}", ins=[], outs=[]
                        )
                        nop.engine = inst.engine
                        nop.sync_info = mybir.SyncInfo(on_wait=[w], on_update=[])
                        nops.append(nop)
                    si.on_wait = keep
                    il[i:i] = nops
                    i += len(nops)
                    n_split += 1
                i += 1
    return n_split


def _build_program(chunk_expert):
    """Bass program for one core: segmented matmul over pre-sorted xT.

    chunk_expert: tuple of expert ids, one per 128-token chunk (static).

    W-stationary orientation: fp32 matmuls re-load their stationary operand
    on every instruction (no standalone ldweights for fp32), so the moving
    operand is made as wide as possible (up to 512 tokens) to amortize it.
    Output is pixel-major ysT [P, ntot]; the host transposes it back.
    """
    ntot = len(chunk_expert) * CHUNK
    assert ntot % 1024 == 0
    # groups of GROUP tokens, with an optional 1024-token tail
    groups = []
    pos = 0
    while pos < ntot:
        gl = min(GROUP, ntot - pos)
        groups.append((pos, gl))
        pos += gl

    nc = bass.Bass(trn_type="TRN2")
    dt = mybir.dt
    xT = nc.declare_dram_parameter("xT", [D, ntot], dt.float32, isOutput=False)
    Wp = nc.declare_dram_parameter("Wp", [128, E * 2 * P], dt.float32, isOutput=False)
    bT = nc.declare_dram_parameter("bT", [P, E], dt.float32, isOutput=False)
    ysT = nc.declare_dram_parameter("ysT", [P, ntot], dt.float32, isOutput=True)

    with tile.TileContext(nc) as tc:
        with (
            tc.tile_pool(name="consts", bufs=1) as consts,
            tc.tile_pool(name="xtp", bufs=6) as xtp,
            tc.tile_pool(name="yp", bufs=4) as yp,
            tc.tile_pool(name="ps", bufs=8, space="PSUM") as ps,
        ):
            wt = consts.tile([128, E * 2 * P], dt.float32)
            nc.sync.dma_start(wt[:], Wp[:])
            bt = consts.tile([P, E], dt.float32)
            nc.sync.dma_start(bt[:], bT[:])

            for gi, (gof, gl) in enumerate(groups):
                xt0 = xtp.tile([128, GROUP], dt.float32, tag="xt0")
                xt1 = xtp.tile([128, GROUP], dt.float32, tag="xt1")
                # split the first group's loads so the PE can start after the
                # first 512-column piece lands instead of the whole MiB
                step = 512 if gi == 0 else gl
                for s in range(0, gl, step):
                    nc.sync.dma_start(
                        xt0[:, s : s + step], xT[0:128, gof + s : gof + s + step]
                    )
                    nc.sync.dma_start(
                        xt1[:, s : s + step], xT[128:256, gof + s : gof + s + step]
                    )

                yts = yp.tile([P, GROUP], dt.float32, tag="yts")
                # runs of equal expert within 512-aligned blocks (moving
                # operand / PSUM bank limit for fp32 is 512)
                base = gof // CHUNK
                runs = []
                for blk in range(gl // 512):
                    start = blk * 4
                    for c in range(start, start + 4):
                        e = chunk_expert[base + c]
                        if runs and runs[-1][0] == e and runs[-1][1] + runs[-1][2] == c and c != start:
                            runs[-1][2] += 1
                        else:
                            runs.append([e, c, 1])
                for e, c0, ln in runs:
                    off = c0 * CHUNK
                    n = ln * CHUNK
                    pt = ps.tile([P, n], dt.float32, tag="pt")
                    nc.tensor.matmul(
                        pt[:],
                        lhsT=wt[:, (e * 2 + 0) * P : (e * 2 + 1) * P],
                        rhs=xt0[:, off : off + n],
                        start=True,
                        stop=False,
                    )
                    nc.tensor.matmul(
                        pt[:],
                        lhsT=wt[:, (e * 2 + 1) * P : (e * 2 + 2) * P],
                        rhs=xt1[:, off : off + n],
                        start=False,
                        stop=True,
                    )
                    # bias add doubles as the PSUM->SBUF copy
                    nc.vector.tensor_scalar_add(
                        yts[:, off : off + n], pt[:], bt[:, e : e + 1]
                    )
                nc.scalar.dma_start(ysT[:, gof : gof + gl], yts[:, :gl])

    return nc


_cache = {"key": None, "nc": None}
last_exec_time_ns = None


def kernel(x, W, b, block_indices):
    global last_exec_time_ns
    x = np.asarray(x, dtype=np.float32)
    W = np.asarray(W, dtype=np.float32)
    b = np.asarray(b, dtype=np.float32)
    sel = np.asarray(block_indices).astype(np.int64)

    x_sh = x.reshape(N_CORES, N_SHARD, D)
    sel_sh = sel.reshape(N_CORES, N_SHARD)

    # per-(core, expert) token lists, grouped by expert via stable argsort
    counts = np.zeros((N_CORES, E), dtype=np.int64)
    ids = []
    for c in range(N_CORES):
        order = np.argsort(sel_sh[c], kind="stable")
        se = sel_sh[c][order]
        bounds = np.searchsorted(se, np.arange(E + 1))
        ids.append([order[bounds[e] : bounds[e + 1]] for e in range(E)])
        counts[c] = bounds[1:] - bounds[:-1]

    # shared static segment layout: capacity per expert = max over cores,
    # rounded up to CHUNK; total rounded up to GROUP
    caps = ((counts.max(axis=0) + CHUNK - 1) // CHUNK * CHUNK).astype(np.int64)
    ntot = int(((caps.sum() + 1023) // 1024) * 1024)
    caps[E - 1] += ntot - caps.sum()
    offs = np.concatenate([[0], np.cumsum(caps)])

    chunk_expert = []
    for e in range(E):
        chunk_expert += [e] * (int(caps[e]) // CHUNK)
    chunk_expert = tuple(chunk_expert)

    if _cache["key"] != chunk_expert:
        nc = _build_program(chunk_expert)
        _split_multi_waits(nc)
        _cache["nc"] = nc
        _cache["key"] = chunk_expert

    # weights: [E, D, P] -> [128, E*2*P] tiles (K-half h of expert e at
    # columns (e*2+h)*P); bias transposed to per-partition columns [P, E]
    Wp = np.ascontiguousarray(
        W.reshape(E, 2, 128, P).transpose(2, 0, 1, 3).reshape(128, E * 2 * P)
    )
    bT = np.ascontiguousarray(b.T)

    in_maps = []
    for c in range(N_CORES):
        # padded sorted order; pad slots replay token 0 (results discarded)
        po = np.zeros(ntot, dtype=np.int64)
        for e in range(E):
            po[offs[e] : offs[e] + counts[c, e]] = ids[c][e]
        xT = np.ascontiguousarray(x_sh[c][po].T)
        in_maps.append({"xT": xT, "Wp": Wp, "bT": bT})

    trace = bool(os.environ.get("BASS_KERNEL_TRACE"))
    res = run_bass_kernel_spmd(
        _cache["nc"], in_maps, list(range(N_CORES)), trace=trace
    )
    last_exec_time_ns = res.exec_time_ns

    out = np.empty((N_CORES, N_SHARD, P), dtype=np.float32)
    for c in range(N_CORES):
        ys = np.ascontiguousarray(res.results[c]["ysT"].T)
        flat = out[c]
        for e in range(E):
            flat[ids[c][e]] = ys[offs[e] : offs[e] + counts[c, e]]
    return out.reshape(B, T, P)
